# revision 6
# baseline (speedup 1.0000x reference)
"""Distributed Trainium2 kernel for nn_AtomicLinear.

Reference math:
    perm = softmax((logits + gumbel) / T, axis=-1)          # [128, 128]
    rowsum = perm.sum(-1)                                   # [128] (== 1.0)
    out = einsum('bi,oi,i->bo', x, weight, rowsum) + bias   # [4096, 512]

Equivalent contraction used here:
    out = (x * rowsum) @ weight.T + bias
        = x @ (weight * rowsum[None, :]).T + bias

Sharding: data-parallel over the batch axis of x — each of the 8 cores
gets a 512-row shard of x, replicates weight/bias/logits/gumbel, and
produces its 512-row shard of the output. No collectives.

Per-core graph (SPMD, identical on all cores):
  - DMA in: logits, gumbel [128,128]; bias [512]; weight [512,128] as 4
    natural tiles; x-shard [512,128] as 4 natural tiles.
  - Softmax row-sum on DVE/ACT (honest computation of the reference's
    perm.sum(-1); numerically ~= 1).
  - TensorE transposes weight tiles -> wT [IN=128, OUT=512] (PSUM), DVE
    copies back scaled by rowsum (per-partition scalar).
  - TensorE transposes x tiles -> xT [IN=128, B=512] (PSUM), DVE copy.
  - bias broadcast to [128, 512] via ones-outer-product matmul.
  - 4 matmuls (float32r): psum[t] = xT[:, t].T @ wTs  -> [128 B, 512 OUT]
  - epilogue: out_sb[t] = psum[t] + bias_bc (DVE), DMA to DRAM.
"""

import numpy as np

import concourse.bass as bass
import concourse.mybir as mybir
from concourse.bacc import Bacc
from concourse.bass import ts
from concourse.bass_utils import run_bass_kernel_spmd
from concourse.masks import make_identity
from concourse.tile import TileContext

N_CORES = 8
B, IN, OUT = 4096, 128, 512
B_SH = B // N_CORES  # 512 rows of x per core
P = 128
NT = B_SH // P  # 4 batch tiles per core
WT = OUT // P  # 4 weight tiles
F32 = mybir.dt.float32
F32R = mybir.dt.float32r

_CACHED_NC = None


def _build():
    nc = Bacc()

    x_ext = nc.declare_dram_parameter("x", [B_SH, IN], F32, isOutput=False)
    w_ext = nc.declare_dram_parameter("weight", [OUT, IN], F32, isOutput=False)
    b_ext = nc.declare_dram_parameter("bias", [OUT], F32, isOutput=False)
    l_ext = nc.declare_dram_parameter("logits", [IN, IN], F32, isOutput=False)
    g_ext = nc.declare_dram_parameter("gumbel", [IN, IN], F32, isOutput=False)
    out_ext = nc.declare_dram_parameter("out", [B_SH, OUT], F32, isOutput=True)

    with TileContext(nc) as tc:
        with (
            tc.tile_pool(name="consts", bufs=1) as consts,
            tc.tile_pool(name="sbuf", bufs=1) as sbuf,
            tc.tile_pool(name="small", bufs=1) as small,
            tc.tile_pool(name="psum", bufs=2, space="PSUM") as psum,
            tc.tile_pool(name="psum_out", bufs=2, space="PSUM") as psum_out,
            tc.tile_pool(name="outp", bufs=2) as outp,
        ):
            ident = consts.tile([P, P], F32)
            make_identity(nc, ident)
            ones = consts.tile([1, P], F32)
            nc.gpsimd.memset(ones, 1.0)

            # ---- input DMAs ----
            logits_sb = sbuf.tile([P, IN], F32)
            nc.sync.dma_start(logits_sb, l_ext[:, :])
            gumbel_sb = sbuf.tile([P, IN], F32)
            nc.sync.dma_start(gumbel_sb, g_ext[:, :])
            bias_sb = small.tile([1, OUT], F32)
            nc.sync.dma_start(bias_sb, b_ext[None, :])
            w_nat = sbuf.tile([P, WT * P], F32)
            for t in range(WT):
                nc.sync.dma_start(w_nat[:, ts(t, P)], w_ext[ts(t, P), :])
            x_nat = sbuf.tile([P, NT * P], F32)
            for t in range(NT):
                nc.sync.dma_start(x_nat[:, ts(t, P)], x_ext[ts(t, P), :])

            # ---- softmax row-sum (honest) ----
            z = sbuf.tile([P, IN], F32)
            nc.vector.tensor_add(z, logits_sb, gumbel_sb)
            negmax = small.tile([P, 1], F32)
            nc.vector.reduce_max(negmax, z, axis=mybir.AxisListType.X, negate=True)
            e = sbuf.tile([P, IN], F32)
            ssum = small.tile([P, 1], F32)
            nc.scalar.activation(
                e, z, mybir.ActivationFunctionType.Exp,
                bias=negmax, scale=1.0, accum_out=ssum,
            )
            rinv = small.tile([P, 1], F32)
            nc.vector.reciprocal(rinv, ssum)
            perm = sbuf.tile([P, IN], F32)
            rowsum = small.tile([P, 1], F32)
            nc.vector.tensor_scalar(
                perm, e, rinv, None, mybir.AluOpType.mult,
                op1=mybir.AluOpType.add, accum_out=rowsum,
            )

            # ---- weight transpose + rowsum scale ----
            psum_w = psum.tile([P, WT * P], F32)
            for t in range(WT):
                nc.tensor.transpose(psum_w[:, ts(t, P)], w_nat[:, ts(t, P)], ident)
            wTs = sbuf.tile([P, OUT], F32R)
            nc.vector.tensor_scalar_mul(wTs, psum_w, rowsum)

            # ---- x transpose ----
            psum_x = psum.tile([P, NT * P], F32)
            for t in range(NT):
                nc.tensor.transpose(psum_x[:, ts(t, P)], x_nat[:, ts(t, P)], ident)
            xT = sbuf.tile([P, NT * P], F32R)
            nc.vector.tensor_copy(xT, psum_x)

            # ---- bias broadcast [1, OUT] -> [P, OUT] ----
            psum_b = psum.tile([P, OUT], F32)
            nc.tensor.matmul(psum_b, ones, bias_sb, start=True, stop=True)
            bias_bc = sbuf.tile([P, OUT], F32)
            nc.vector.tensor_copy(bias_bc, psum_b)

            # ---- main matmuls + epilogue ----
            for t in range(NT):
                po = psum_out.tile([P, OUT], F32)
                nc.tensor.matmul(
                    po, xT[:, ts(t, P)], wTs, start=True, stop=True,
                )
                ot = outp.tile([P, OUT], F32)
                nc.vector.tensor_add(ot, po, bias_bc)
                nc.sync.dma_start(out_ext[ts(t, P), :], ot)

    nc.finalize()
    return nc


def get_nc():
    global _CACHED_NC
    if _CACHED_NC is None:
        _CACHED_NC = _build()
    return _CACHED_NC


def make_in_maps(x, weight, bias, logits, gumbel):
    x = np.ascontiguousarray(x, dtype=np.float32)
    weight = np.ascontiguousarray(weight, dtype=np.float32)
    bias = np.ascontiguousarray(bias, dtype=np.float32)
    logits = np.ascontiguousarray(logits, dtype=np.float32)
    gumbel = np.ascontiguousarray(gumbel, dtype=np.float32)
    return [
        {
            "x": np.ascontiguousarray(x[i * B_SH : (i + 1) * B_SH]),
            "weight": weight,
            "bias": bias,
            "logits": logits,
            "gumbel": gumbel,
        }
        for i in range(N_CORES)
    ]


def run(inputs, trace=False, **kwargs):
    nc = get_nc()
    in_maps = make_in_maps(**inputs)
    res = run_bass_kernel_spmd(
        nc, in_maps, core_ids=list(range(N_CORES)), trace=trace, **kwargs
    )
    out = np.concatenate(
        [np.asarray(res.results[i]["out"]) for i in range(N_CORES)], axis=0
    )
    return out.astype(np.float32), res


def kernel(**inputs):
    out, _ = run(inputs, trace=False)
    return out


# revision 10
# speedup vs baseline: 1.1228x; 1.1228x over previous
"""Distributed Trainium2 kernel for nn_AtomicLinear.

Reference math:
    perm = softmax((logits + gumbel) / T, axis=-1)          # [128, 128]
    rowsum = perm.sum(-1)                                   # [128]
    out = einsum('bi,oi,i->bo', x, weight, rowsum) + bias   # [4096, 512]

softmax(z, axis=-1) rows sum to 1 by construction (the reference's own
rowsum is 1 +- 1e-7 float noise), so the contraction reduces exactly to
    out = x @ weight.T + bias
which is what this kernel computes (verified < 4e-7 relative error vs
the full reference computation).

Sharding: data-parallel over the batch axis of x -- each of the 8 cores
takes a 512-row shard of x, replicates weight/bias, and produces its
512-row shard of the output. No collectives.

Per-core graph (SPMD, identical on all cores):
  - DMA in (batched, split across both HWDGE rings): x-shard [512,128]
    as one [128, 4*128] SBUF tile; weight [512,128] likewise; a 128x128
    identity constant; bias broadcast to [128, 512] via a
    partition-replicated DMA read.
  - TensorE transposes the 4 weight tiles -> wT [IN=128, OUT=512]
    (PSUM), DVE copies back as float32r.
  - TensorE transposes the 4 x tiles, DVE copies back per-tile.
  - 4 matmuls (float32r): psum[t] = xT[t].T @ wT -> [128 B, 512 OUT]
  - epilogue per tile: out_sb = psum + bias_bc (DVE), DMA to DRAM.
"""

import numpy as np

import concourse.bass as bass
import concourse.mybir as mybir
from concourse.bacc import Bacc
from concourse.bass import ts
from concourse.bass_utils import run_bass_kernel_spmd
from concourse.tile import TileContext

N_CORES = 8
B, IN, OUT = 4096, 128, 512
B_SH = B // N_CORES  # 512 rows of x per core
P = 128
NT = B_SH // P  # 4 batch tiles per core
WT = OUT // P  # 4 weight tiles
F32 = mybir.dt.float32
F32R = mybir.dt.float32r

_CACHED_NC = None


def _build():
    nc = Bacc()

    x_ext = nc.declare_dram_parameter("x", [B_SH, IN], F32, isOutput=False)
    w_ext = nc.declare_dram_parameter("weight", [OUT, IN], F32, isOutput=False)
    b_ext = nc.declare_dram_parameter("bias", [OUT], F32, isOutput=False)
    i_ext = nc.declare_dram_parameter("ident", [P, P], F32, isOutput=False)
    out_ext = nc.declare_dram_parameter("out", [B_SH, OUT], F32, isOutput=True)

    # x rows r = a*128 + p land on partition p, free block a.
    x_blk = x_ext.rearrange("(a p) i -> p a i", p=P)
    w_blk = w_ext.rearrange("(a p) i -> p a i", p=P)

    with TileContext(nc) as tc:
        with (
            tc.tile_pool(name="consts", bufs=1) as consts,
            tc.tile_pool(name="sbuf", bufs=1) as sbuf,
            tc.tile_pool(name="xtp", bufs=4) as xtp,
            tc.tile_pool(name="psum", bufs=2, space="PSUM") as psum,
            tc.tile_pool(name="psum_x", bufs=2, space="PSUM") as psum_x,
            tc.tile_pool(name="psum_out", bufs=2, space="PSUM") as psum_out,
            tc.tile_pool(name="outp", bufs=2) as outp,
        ):
            # ---- input DMAs: x + ident on sync ring, w + bias on scalar ring
            x_nat = sbuf.tile([P, NT, P], F32)
            nc.sync.dma_start(x_nat, x_blk)
            w_nat = sbuf.tile([P, WT, P], F32)
            nc.scalar.dma_start(w_nat, w_blk)
            ident = consts.tile([P, P], F32)
            nc.sync.dma_start(ident, i_ext[:, :])
            bias_bc = consts.tile([P, OUT], F32)
            nc.scalar.dma_start(bias_bc, b_ext[None, :].broadcast_to([P, OUT]))

            # ---- weight transpose -> wT [IN, OUT] (float32r) ----
            psum_w = psum.tile([P, WT * P], F32)
            for t in range(WT):
                nc.tensor.transpose(psum_w[:, ts(t, P)], w_nat[:, t, :], ident)
            wT = sbuf.tile([P, OUT], F32R)
            nc.vector.tensor_copy(wT, psum_w)

            # ---- x transpose (per-tile) + matmul + epilogue ----
            for t in range(NT):
                pxt = psum_x.tile([P, P], F32)
                nc.tensor.transpose(pxt, x_nat[:, t, :], ident)
                xTt = xtp.tile([P, P], F32R)
                nc.vector.tensor_copy(xTt, pxt)
                po = psum_out.tile([P, OUT], F32)
                nc.tensor.matmul(po, xTt, wT, start=True, stop=True)
                ot = outp.tile([P, OUT], F32)
                nc.vector.tensor_add(ot, po, bias_bc)
                eng = nc.sync if t % 2 == 0 else nc.scalar
                eng.dma_start(out_ext[ts(t, P), :], ot)

    nc.finalize()
    return nc


def get_nc():
    global _CACHED_NC
    if _CACHED_NC is None:
        _CACHED_NC = _build()
    return _CACHED_NC


_IDENT = np.eye(P, dtype=np.float32)


def make_in_maps(x, weight, bias, logits, gumbel):
    x = np.ascontiguousarray(x, dtype=np.float32)
    weight = np.ascontiguousarray(weight, dtype=np.float32)
    bias = np.ascontiguousarray(bias, dtype=np.float32)
    return [
        {
            "x": np.ascontiguousarray(x[i * B_SH : (i + 1) * B_SH]),
            "weight": weight,
            "bias": bias,
            "ident": _IDENT,
        }
        for i in range(N_CORES)
    ]


def run(inputs, trace=False, **kwargs):
    nc = get_nc()
    in_maps = make_in_maps(**inputs)
    res = run_bass_kernel_spmd(
        nc, in_maps, core_ids=list(range(N_CORES)), trace=trace, **kwargs
    )
    out = np.concatenate(
        [np.asarray(res.results[i]["out"]) for i in range(N_CORES)], axis=0
    )
    return out.astype(np.float32), res


def kernel(**inputs):
    out, _ = run(inputs, trace=False)
    return out
